# revision 7
# baseline (speedup 1.0000x reference)
"""GQA kernel for Trainium2, 8 NeuronCores.

Sharding: DP=2 over batch x TP=4 over KV-head groups (2 KV heads = 8 query
heads per core).  Each core computes attention for its heads on its batch
element and a partial output projection (wo rows for its heads); the host sums
the 4 TP partials per batch element.

Model dims (hardcoded): bsz=2, seq=2048, dim=2048, 32 q-heads, 8 kv-heads,
head_dim=64.

Per-core dataflow (all fp32 storage; matmuls run as float32r):
  phase 1: PE-transpose x[b] -> XT [d, s]; QT[h,s] = wq_p.T @ XT (row-pair
           layout: partition 0-63 = kv-group a=0, 64-127 = a=1 for the same
           rep index r); K/V projection fused (rhs = [wk|wv], N=256); K
           transposed on PE into KT [hk, s]; V kept natural [s, hv].
  phase 2: per (q-block 512, k-tile 128, r): row-packed score matmuls
           scoresT[k,q] (contraction = head_dim 64, two kv groups packed via
           tile_position rows), exp on ScalarE (scale=1/8 fused; no
           max-subtraction -- scores are O(+-5)), col-packed PV matmuls
           accumulate attnT[hd,q] in PSUM, softmax denominators via
           ones-matmul 4-col-packs into 2 PSUM banks, then normalize at
           eviction (reciprocal + broadcast multiply).
  phase 3: Y_partial = A.T-chunks @ wo_p (row-permuted on host so chunks are
           contiguous), streamed to DRAM.
"""

import os
from contextlib import ExitStack

import numpy as np

BSZ, SEQ, DIM = 2, 2048, 2048
NH, NKV, HD = 32, 8, 64
P = 128
SCALE = 1.0 / 8.0  # 1/sqrt(64)

_CACHE: dict = {}


def build_nc():
    import concourse.bacc as bacc
    import concourse.mybir as mybir
    import concourse.tile as tile
    from concourse.masks import make_identity

    F32 = mybir.dt.float32
    F32R = mybir.dt.float32r
    EXP = mybir.ActivationFunctionType.Exp
    use_f32r = os.environ.get("GQA_MM_FP32", "0") != "1"

    def mm(ap):
        return ap.bitcast(F32R) if use_f32r else ap

    nc = bacc.Bacc("TRN2", target_bir_lowering=False, debug=False, num_devices=8)

    x_d = nc.dram_tensor("x", [SEQ, DIM], F32, kind="ExternalInput").ap()
    wq_d = nc.dram_tensor("wq", [DIM, 512], F32, kind="ExternalInput").ap()
    wkv_d = nc.dram_tensor("wkv", [DIM, 256], F32, kind="ExternalInput").ap()
    wo_d = nc.dram_tensor("wo", [512, DIM], F32, kind="ExternalInput").ap()
    y_d = nc.dram_tensor("y", [SEQ, DIM], F32, kind="ExternalOutput").ap()

    with tile.TileContext(nc) as tc, ExitStack() as ctx:
        persist = ctx.enter_context(tc.tile_pool(name="persist", bufs=1))
        xin_pool = ctx.enter_context(tc.tile_pool(name="xin", bufs=2))
        xt_pool = ctx.enter_context(tc.tile_pool(name="xt", bufs=1))
        pt_pool = ctx.enter_context(tc.tile_pool(name="pt", bufs=10))
        sb_small = ctx.enter_context(tc.tile_pool(name="small", bufs=4))
        ysb_pool = ctx.enter_context(tc.tile_pool(name="ysb", bufs=2))
        ps_a = ctx.enter_context(tc.tile_pool(name="ps_a", bufs=2, space="PSUM"))
        ps_b = ctx.enter_context(tc.tile_pool(name="ps_b", bufs=2, space="PSUM"))
        ps_att = ctx.enter_context(tc.tile_pool(name="ps_att", bufs=4, space="PSUM"))

        # --- persistent tiles -------------------------------------------------
        wq_sb = persist.tile([P, 16, 512], F32, tag="wq")
        nc.sync.dma_start(out=wq_sb[:], in_=wq_d.rearrange("(t p) h -> p t h", p=P))
        wkv_sb = persist.tile([P, 16, 256], F32, tag="wkv")
        nc.sync.dma_start(out=wkv_sb[:], in_=wkv_d.rearrange("(t p) h -> p t h", p=P))

        qt_sb = [persist.tile([P, SEQ], F32, tag=f"qt{r}", name=f"qt{r}") for r in range(4)]
        kt_sb = persist.tile([P, SEQ], F32, tag="kt")
        v_sb = persist.tile([P, 16, 128], F32, tag="v")
        # attention output reuses the QT tiles: qt_sb[r][:, q-block] is dead
        # once the q-block's score matmuls are done, and the normalized
        # attnT has exactly the same layout (Tile handles the WAR dep).
        at_sb = qt_sb

        ident = persist.tile([P, P], F32, tag="ident")
        make_identity(nc, ident[:])
        ones64 = persist.tile([P, 64], F32, tag="ones")
        nc.vector.memset(ones64[:], 1.0)

        # --- phase 1: transpose + projections --------------------------------
        for sb in range(4):
            xt = xt_pool.tile([P, 16, 512], F32, tag="xt")
            for stl in range(4):
                st = sb * 4 + stl
                xin = xin_pool.tile([P, DIM], F32, tag="xin")
                nc.sync.dma_start(out=xin[:], in_=x_d[st * P:(st + 1) * P, :])
                for dg in range(4):
                    tp = ps_a.tile([P, 512], F32, tag="w")
                    for j in range(4):
                        di = dg * 4 + j
                        nc.tensor.matmul(
                            tp[:, j * P:(j + 1) * P],
                            xin[:, di * P:(di + 1) * P],
                            ident[:],
                            is_transpose=True,
                            start=(j == 0),
                            stop=(j == 3),
                        )
                    nc.vector.tensor_copy(
                        xt[:, dg * 4:(dg + 1) * 4, stl * P:(stl + 1) * P],
                        tp[:].rearrange("p (a b) -> p a b", a=4),
                    )
            # QT for this s-block
            for r in range(4):
                qp = ps_b.tile([P, 512], F32, tag="b")
                for di in range(16):
                    nc.tensor.matmul(
                        qp[:],
                        mm(wq_sb[:, di, r * P:(r + 1) * P]),
                        mm(xt[:, di, :]),
                        start=(di == 0),
                        stop=(di == 15),
                    )
                nc.vector.tensor_copy(qt_sb[r][:, sb * 512:(sb + 1) * 512], qp[:])
            # K/V for this s-block
            ktps = ps_att.tile([P, 512], F32, tag="att")
            for stl in range(4):
                st = sb * 4 + stl
                kvp = ps_b.tile([P, 256], F32, tag="b")
                for di in range(16):
                    nc.tensor.matmul(
                        kvp[:],
                        mm(xt[:, di, stl * P:(stl + 1) * P]),
                        mm(wkv_sb[:, di, :]),
                        start=(di == 0),
                        stop=(di == 15),
                    )
                nc.vector.tensor_copy(v_sb[:, st, :], kvp[:, 128:256])
                kt_tmp = sb_small.tile([P, P], F32, tag="ktmp", bufs=2)
                nc.vector.tensor_copy(kt_tmp[:], kvp[:, 0:128])
                nc.tensor.matmul(
                    ktps[:, stl * P:(stl + 1) * P],
                    kt_tmp[:],
                    ident[:],
                    is_transpose=True,
                    start=(stl == 0),
                    stop=(stl == 3),
                )
            nc.vector.tensor_copy(kt_sb[:, sb * 512:(sb + 1) * 512], ktps[:])

        # --- phase 2: attention ----------------------------------------------
        for qi in range(4):
            q0 = qi * 512
            att_ps = [ps_att.tile([P, 512], F32, tag="att", name=f"attps{qi}_{i}") for i in range(4)]
            den_ps = [ps_b.tile([P, 512], F32, tag="b", name=f"denps{qi}_{i}") for i in range(2)]
            for k0 in range(16):
                pts = []
                for r in range(4):
                    s0 = ps_a.tile([P, 512], F32, tag="w")
                    s1 = ps_a.tile([P, 512], F32, tag="w")
                    nc.tensor.matmul(
                        s0[:],
                        mm(kt_sb[0:64, k0 * P:(k0 + 1) * P]),
                        mm(qt_sb[r][0:64, q0:q0 + 512]),
                        start=True, stop=True,
                        tile_position=(0, 0),
                    )
                    nc.tensor.matmul(
                        s1[:],
                        mm(kt_sb[64:128, k0 * P:(k0 + 1) * P]),
                        mm(qt_sb[r][64:128, q0:q0 + 512]),
                        start=True, stop=True,
                        tile_position=(64, 0),
                    )
                    p0 = pt_pool.tile([P, 512], F32, tag="pt")
                    p1 = pt_pool.tile([P, 512], F32, tag="pt")
                    nc.scalar.activation(p0[:], s0[:], EXP, scale=SCALE)
                    nc.scalar.activation(p1[:], s1[:], EXP, scale=SCALE)
                    nc.tensor.matmul(
                        att_ps[r][0:64, :],
                        mm(v_sb[:, k0, 0:64]),
                        mm(p0[:]),
                        start=(k0 == 0), stop=(k0 == 15),
                        tile_position=(0, 0), skip_group_check=True,
                    )
                    nc.tensor.matmul(
                        att_ps[r][64:128, :],
                        mm(v_sb[:, k0, 64:128]),
                        mm(p1[:]),
                        start=(k0 == 0), stop=(k0 == 15),
                        tile_position=(0, 64), skip_group_check=True,
                    )
                    pts.append((p0, p1))
                for a in range(2):
                    for r in range(4):
                        nc.tensor.matmul(
                            den_ps[a][r * 32:(r + 1) * 32, :],
                            mm(ones64[:, 0:32]),
                            mm(pts[r][a][:]),
                            start=(k0 == 0), stop=(k0 == 15),
                            tile_position=(0, r * 32), skip_group_check=True,
                        )
            # reciprocal of the 8 denominator rows (one partition each, kept
            # 32-aligned so they can feed K=1 ones-matmuls that replicate them
            # across 64 partitions -- all-on-chip partition broadcast).
            rda = sb_small.tile([P, 512], F32, tag="rdena", bufs=1, name=f"rda{qi}")
            rdb = sb_small.tile([P, 512], F32, tag="rdenb", bufs=1, name=f"rdb{qi}")
            for r in range(4):
                nc.vector.reciprocal(rda[32 * r:32 * r + 1, :],
                                     den_ps[0][32 * r:32 * r + 1, :])
                nc.vector.reciprocal(rdb[32 * r:32 * r + 1, :],
                                     den_ps[1][32 * r:32 * r + 1, :])
            for r in range(4):
                bcr = ps_a.tile([P, 512], F32, tag="w", name=f"bcr{qi}_{r}")
                nc.tensor.matmul(
                    bcr[0:64, :], ones64[32 * r:32 * r + 1, :],
                    rda[32 * r:32 * r + 1, :],
                    start=True, stop=True, tile_position=(32 * r, 0),
                    skip_group_check=True)
                nc.tensor.matmul(
                    bcr[64:128, :], ones64[32 * r:32 * r + 1, :],
                    rdb[32 * r:32 * r + 1, :],
                    start=True, stop=True, tile_position=(32 * r, 64),
                    skip_group_check=True)
                bc = sb_small.tile([P, 512], F32, tag="bc", bufs=2)
                nc.vector.tensor_copy(bc[:], bcr[:])
                nc.vector.tensor_mul(at_sb[r][:, q0:q0 + 512], att_ps[r][:], bc[:])

        # --- phase 3: output projection --------------------------------------
        wo_r = wo_d.rearrange("(r p) n -> p r n", p=P)
        for nb in range(4):
            wo_sb = ysb_pool.tile([P, 4, 512], F32, tag="wo", name=f"wo{nb}", bufs=2)
            nc.sync.dma_start(out=wo_sb[:], in_=wo_r[:, :, nb * 512:(nb + 1) * 512])
            for st in range(16):
                wp = ps_a.tile([P, 512], F32, tag="w")
                for r in range(4):
                    nc.tensor.matmul(
                        wp[:],
                        mm(at_sb[r][:, st * P:(st + 1) * P]),
                        mm(wo_sb[:, r, :]),
                        start=(r == 0), stop=(r == 3),
                    )
                ysb = ysb_pool.tile([P, 512], F32, tag="y", bufs=3)
                nc.vector.tensor_copy(ysb[:], wp[:])
                nc.sync.dma_start(
                    out=y_d[st * P:(st + 1) * P, nb * 512:(nb + 1) * 512], in_=ysb[:])

    nc.compile()
    return nc


def make_in_maps(x, wq, wk, wv, wo):
    x = np.ascontiguousarray(np.asarray(x, dtype=np.float32))
    wq = np.ascontiguousarray(np.asarray(wq, dtype=np.float32))
    wk = np.ascontiguousarray(np.asarray(wk, dtype=np.float32))
    wv = np.ascontiguousarray(np.asarray(wv, dtype=np.float32))
    wo = np.ascontiguousarray(np.asarray(wo, dtype=np.float32))
    in_maps = []
    for c in range(8):
        b, t = divmod(c, 4)
        g0 = 2 * t
        perm = np.array(
            [(4 * (g0 + a) + r) * 64 + j
             for r in range(4) for a in range(2) for j in range(64)],
            dtype=np.int64,
        )
        in_maps.append({
            "x": x[b],
            "wq": np.ascontiguousarray(wq[:, perm]),
            "wkv": np.ascontiguousarray(np.concatenate(
                [wk[:, g0 * 64:(g0 + 2) * 64], wv[:, g0 * 64:(g0 + 2) * 64]],
                axis=1)),
            "wo": np.ascontiguousarray(wo[perm, :]),
        })
    return in_maps


def kernel(x, wq, wk, wv, wo):
    from concourse.bass_utils import run_bass_kernel_spmd

    if "nc" not in _CACHE:
        _CACHE["nc"] = build_nc()
    nc = _CACHE["nc"]
    in_maps = make_in_maps(x, wq, wk, wv, wo)
    results = run_bass_kernel_spmd(nc, in_maps, list(range(8))).results
    y = np.empty((BSZ, SEQ, DIM), np.float32)
    for b in range(2):
        y[b] = (results[4 * b]["y"] + results[4 * b + 1]["y"]
                + results[4 * b + 2]["y"] + results[4 * b + 3]["y"])
    return y
